# revision 23
# baseline (speedup 1.0000x reference)
"""Inverse Haar DWT2 (pywt 'haar' idwt2 convention) on 8 Trainium2 cores.

Input  x: [16, 256, 128, 128] f32 — 4 stacked subbands (LL|LH|HL|HH) of 64
channels each.  Output: [16, 64, 256, 256] f32.

Sharding: batch dim (16) split across 8 cores, 2 batches per core.  The
transform is elementwise per (batch, channel) — no communication.

Per-core kernel (x_loc [2, 256, 128, 128] -> y_loc [2, 64, 256, 256]):
SBUF partition dim = (batch, channel) = 2*64 = 128; free dim = a chunk of
hc input rows * 128 cols (hc=8 steady state, tapered 4/2/2 at the end so
the post-last-load pipeline drain is short).  Per tile:
  - 4 DMAs (one per subband) load T [128p, band*hc*128] on the SP HW-DGE
    ring; DRAM-side inner runs are hc*512B contiguous
  - pre-scale (ScalarE/ACT): T[LL|LH] *= 0.5 in place
  - stage 1 (DVE tensor_tensor): U = (LL/2|HL)+(LH/2|HH) = (p/2|r),
    V = (LL/2|HL)-(LH/2|HH) = (q/2|s)
  - stage 2 (DVE scalar_tensor_tensor): out[2i+rr, 2j+s] =
    (R * +-0.5) + P/2 — the remaining Haar 0.5 is folded into the stt
    scalar, so there is no separate full-tile scale pass
  - 1 DMA stores OUT on the ACT HW-DGE ring (separate ring from loads so
    a compute-gated store never head-of-line-blocks loads), emitted one
    iteration late so it also never blocks the next prescale on the ACT
    instruction queue
Engine budget per hc=8 tile: DVE 8.8us, ACT 1.9us, DMA engines 10.0us
each -> DMA-bound.  The 16 SDMA engines stream at a flat ~26 GB/s each
(4-32KB descriptors alike), so the ~161us aggregate transfer time
(67.1 MB per core / ~416 GB/s) is the hard floor; measured ~175us
includes ~9us Tile/runtime preamble and ~4us drain epilogue.

This container's walrus build supports only ONE semaphore wait per
instruction; Tile emits multi-wait instructions (incl. the final drain), so
after TileContext exit we redistribute extra waits onto single-wait NOPs
inserted before the instruction on the same engine.
"""

import numpy as np

import concourse.bass as bass
import concourse.mybir as mybir
from concourse.tile import TileContext
from concourse.bass_utils import run_bass_kernel_spmd

N_CORES = 8
B, C4, H, W = 16, 256, 128, 128
CH = C4 // 4          # 64 output channels
B_LOC = B // N_CORES  # 2 batches per core
HC = 8                # input rows per tile iteration
F32 = mybir.dt.float32


def _split_multi_waits(nc):
    """Move extra semaphore waits onto single-wait NOPs placed immediately
    before the over-subscribed instruction (same engine, so per-engine
    program order is preserved)."""
    n_split = 0
    for f in nc.m.functions:
        for blk in f.blocks:
            il = blk.instructions
            new_list = []
            for inst in il:
                si = getattr(inst, "sync_info", None)
                ow = si.on_wait if si is not None else None
                if ow and len(ow) > 1:
                    extra = list(ow[:-1])
                    del ow[:-1]
                    for w in extra:
                        n_split += 1
                        new_list.append(
                            mybir.InstNoOp(
                                name=f"{inst.name}-waitsplit-{n_split}",
                                engine=inst.engine,
                                sync_info=mybir.SyncInfo(on_wait=[w], on_update=[]),
                            )
                        )
                new_list.append(inst)
            il[:] = new_list
    return n_split


def _build_kernel():
    nc = bass.Bass("TRN2")
    x = nc.dram_tensor("x", [B_LOC, C4, H, W], F32, kind="ExternalInput")
    y = nc.dram_tensor("y", [B_LOC, CH, 2 * H, 2 * W], F32, kind="ExternalOutput")

    # Taper the tail: the last tiles shrink so the post-last-load drain
    # (prescale -> stage1 -> stage2 -> store of the final tile) is short.
    tiles = [(i * HC, HC) for i in range(15)] + [(120, 4), (124, 2), (126, 2)]
    assert sum(hc for _, hc in tiles) == H

    with TileContext(nc) as tc:
        with (
            tc.tile_pool(name="tin", bufs=4) as pin,
            tc.tile_pool(name="tuv", bufs=2) as puv,
            tc.tile_pool(name="tout", bufs=3) as pout,
        ):
            pending_store = None  # (OUT tile, h0, hc) deferred one iteration

            def flush_store(engine=None):
                pOUT, ph0, phc = pending_store
                (engine or nc.scalar).dma_start(
                    out=y[:, :, 2 * ph0 : 2 * ph0 + 2 * phc, :]
                    .rearrange("b c h w -> c b (h w)"),
                    in_=pOUT[:],
                )

            for h0, hc in tiles:
                fb = hc * W  # free elems per band block
                # ---- load: T [p=(c,b)][band][i][w]  (SP HW-DGE ring)
                # partition p = c*2 + b so the DRAM AP's outermost dim has
                # count 64 (the HWDGE engine spray follows the outer source
                # dim; outer count 2 would use only 2 of 16 SDMA engines)
                T = pin.tile([128, 4 * fb], F32, tag="T")
                for band in range(4):
                    nc.sync.dma_start(
                        out=T[:, band * fb : (band + 1) * fb],
                        in_=x[:, band * CH : (band + 1) * CH, h0 : h0 + hc, :]
                        .rearrange("b c h w -> c b (h w)"),
                    )
                # ---- pre-scale (ACT): halve LL|LH in place.  Stage 2 folds
                # the other 0.5 onto the (HL,HH) terms via
                # scalar_tensor_tensor, so no full-tile scale pass is needed.
                nc.scalar.mul(T[:, : 2 * fb], T[:, : 2 * fb], 0.5)
                # ---- deferred store of the PREVIOUS tile.  Emitting it
                # after this iteration's prescale gives the ACT queue a full
                # DVE-iteration of slack before a stage-2-gated store could
                # head-of-line-block the next prescale.
                if pending_store is not None:
                    flush_store()
                # ---- stage 1 (DVE): vertical butterfly
                # band = b1*2 + b0: LL=00 LH=01 HL=10 HH=11
                # in0 = (LL/2, HL) [b0=0], in1 = (LH/2, HH) [b0=1]
                UV = puv.tile([128, 4 * fb], F32, tag="UV")  # [r][U|V][i][w]
                Tb = T[:].rearrange("p (b1 b0 x) -> p b1 b0 x", b1=2, b0=2)
                in0 = Tb[:, :, 0]
                in1 = Tb[:, :, 1]
                UVr = UV[:].rearrange("p (r x) -> p r x", r=2)
                out0 = UVr[:, 0].rearrange("p (pair x) -> p pair x", pair=2)
                out1 = UVr[:, 1].rearrange("p (pair x) -> p pair x", pair=2)
                nc.vector.tensor_add(out=out0, in0=in0, in1=in1)  # p/2 | r
                nc.vector.tensor_sub(out=out1, in0=in0, in1=in1)  # q/2 | s
                # ---- stage 2 (DVE): horizontal butterfly + column interleave
                # out[2i+rr, 2j+s] = P/2 +- R/2 computed as (R*+-0.5) + P/2
                # with scalar_tensor_tensor.  OUT free layout [i][rr][col],
                # col = 2j+s.  Keep every AP at <=2 free dims — 3-free-dim
                # strided DVE ops run ~2x slower.
                OUT = pout.tile([128, 2 * hc * 2 * W], F32, tag="OUT")
                OUTv = OUT[:].rearrange(
                    "p (i r j s) -> p i r j s", i=hc, r=2, j=W, s=2
                )
                UVv = UV[:].rearrange(
                    "p (r pair i w) -> p r pair i w", r=2, pair=2, i=hc
                )
                for rr in range(2):
                    P = UVv[:, rr, 0]  # p/2 (rr=0) or q/2 (rr=1)
                    R = UVv[:, rr, 1]  # r   (rr=0) or s   (rr=1)
                    nc.vector.scalar_tensor_tensor(
                        out=OUTv[:, :, rr, :, 0], in0=R, scalar=0.5, in1=P,
                        op0=mybir.AluOpType.mult, op1=mybir.AluOpType.add,
                    )
                    nc.vector.scalar_tensor_tensor(
                        out=OUTv[:, :, rr, :, 1], in0=R, scalar=-0.5, in1=P,
                        op0=mybir.AluOpType.mult, op1=mybir.AluOpType.add,
                    )
                # ---- store (ACT HW-DGE ring, deferred one iteration so
                # stores never head-of-line-block loads or prescales)
                pending_store = (OUT, h0, hc)
            # flush the final deferred store
            flush_store()

    _split_multi_waits(nc)
    return nc


_NC_CACHE = None


def _get_nc():
    global _NC_CACHE
    if _NC_CACHE is None:
        _NC_CACHE = _build_kernel()
    return _NC_CACHE


def run_sharded(x, trace=False, **kwargs):
    assert x.shape == (B, C4, H, W) and x.dtype == np.float32
    nc = _get_nc()
    in_maps = [
        {"x": np.ascontiguousarray(x[i * B_LOC : (i + 1) * B_LOC])}
        for i in range(N_CORES)
    ]
    res = run_bass_kernel_spmd(
        nc, in_maps, core_ids=list(range(N_CORES)), trace=trace, **kwargs
    )
    out = np.concatenate([r["y"] for r in res.results], axis=0)
    return out, res


def kernel(x):
    out, _ = run_sharded(np.asarray(x))
    return out

